# revision 1
# baseline (speedup 1.0000x reference)
"""Trainium2 Bass kernel for the BAHDANAU+ group-recommendation model.

kernel(**inputs) takes the complete (unsharded) numpy inputs, distributes the
131072-query batch over 8 NeuronCores, runs the Bass kernel SPMD, and returns
the full [B, 1] float32 output.

Architecture (v3):
  Host-side TABLE transforms (query-independent):
    group_tab[g] = user_emb[members[g]] flattened to 96 cols, zero-padded to
    128 cols, bf16  ([500K, 128]).  item_tab = item_emb||genres, bf16,
    [100K, 32].  (Denormalizing the member->user lookup is a table transform,
    same category as the item_emb||genres concat.)

  Sharding: queries are assigned to cores by item-id range (12500 items per
  core) so each core's item ids fit int16 after rebasing; each core receives
  its own 12500-row slice of item_tab.  Within a core, queries are ordered by
  group-table chunk (16 chunks of 32768 rows, int16-indexable), each chunk
  padded to a static NPC positions with dummy (idx 0) queries.

  Gathers use the SWDGE dma_gather ucode (InstDMAGatherAnt): 16 group-chunk
  gathers (NPC indices each, 256B rows) + 4 item gathers (NPOS/4 indices,
  64B rows) per core -- ~20 Pool-engine instructions instead of the 640
  one-index-per-partition indirect DMAs of the baseline (~1.4us each).

  Compute per chunk of TPC tiles (tile = 128 positions):
    merge item cols into gbig; PE-transpose gbig -> gbigT; attention logits
    via matmul (lhsT=gbigT, rhs=attn_W); g = sum_k at_k*mem_k on DVE
    (row-major); PE-transpose g; assemble newGT=[gitT;gT;itT] on 96
    partitions; h = relu(new@W1+b1) via one matmul per tile; y = h@W2 on
    DVE; sigmoid(+b2) on the scalar engine.  All embedding math in bf16.

  Output y is produced in permuted position order; the host scatters it back
  with the inverse permutation.
"""

import sys

sys.path.insert(0, "/opt/trn_rl_repo")

from contextlib import ExitStack

import numpy as np
import ml_dtypes

import concourse.bacc as bacc
import concourse.bass as bass
import concourse.tile as tile
from concourse import library_config, masks, mybir
from concourse.bass_utils import run_bass_kernel_spmd

N_CORES = 8
P = 128
EMB = 32
B = 131_072
NUM_USERS = 1_000_000
NUM_ITEMS = 100_000
NUM_GROUPS = 500_000
N_CHUNKS = 16
CHUNK = 32_768          # group-table rows per chunk (int16 range)
ITEMS_PER_CORE = NUM_ITEMS // N_CORES
N_ITEM_GATHERS = 4
# 32 = gather bare 64B item rows (needs the %256 elem assert relaxed);
# 128 = gather 256B zero-padded rows and merge with an add (always safe).
ITEM_ELEM = 128
# Max indices per dma_gather instruction (must be a multiple of 128; HW
# crashes above 512).  256 measured 6.2ns/idx vs 9.0ns/idx at 512 on HW
# (packet-count threshold in the ucode), so smaller pieces are faster.
MAX_GATHER = 256

F32 = mybir.dt.float32
BF16 = mybir.dt.bfloat16
I16 = mybir.dt.int16
I32 = mybir.dt.int32
MULT = mybir.AluOpType.mult
ADD = mybir.AluOpType.add
AXX = mybir.AxisListType.X


def split_gather(nc, out_tile, tile0, in_ap, idx_tile, pos0, n, elem):
    """Emit dma_gather(s) for `n` positions starting at global position
    `pos0`, splitting into MAX_GATHER-index pieces.  out_tile is the
    [128, nt, elem] dst tile; tile0 = first dst tile-column; idx_tile is
    the [128, npos//16] wrapped index tile."""
    done = 0
    while done < n:
        piece = min(MAX_GATHER, n - done)
        assert piece % P == 0
        p0 = pos0 + done
        nc.gpsimd.dma_gather(
            out_ap=out_tile[:, tile0 + done // P : tile0 + (done + piece) // P, :],
            in_ap=in_ap,
            idxs_ap=idx_tile[:, p0 // 16 : (p0 + piece) // 16],
            num_idxs=piece,
            num_idxs_reg=piece,
            elem_size=elem,
        )
        done += piece


def build(npc):
    """Per-core Bass program for 16 chunks x npc positions (npc % 128 == 0)."""
    assert npc % P == 0
    tpc = npc // P               # tiles per chunk
    nt = N_CHUNKS * tpc          # total tiles
    npos = nt * P

    nc = bacc.Bacc(
        "TRN2",
        target_bir_lowering=False,
        debug=False,
        enable_asserts=False,
    )

    gidx = nc.dram_tensor("gidx", [P, npos // 16], I16, kind="ExternalInput")
    iidx = nc.dram_tensor("iidx", [P, npos // 16], I16, kind="ExternalInput")
    group_tab = nc.dram_tensor("group_tab", [NUM_GROUPS, P], BF16, kind="ExternalInput")
    item_tab = nc.dram_tensor(
        "item_tab", [ITEMS_PER_CORE, ITEM_ELEM], BF16, kind="ExternalInput"
    )
    attn_w_d = nc.dram_tensor("attn_w", [P, 3], BF16, kind="ExternalInput")
    w1_d = nc.dram_tensor("w1", [3 * EMB, 8], BF16, kind="ExternalInput")
    attnb_d = nc.dram_tensor("attnb", [P, 3], F32, kind="ExternalInput")
    b1_d = nc.dram_tensor("b1", [P, 8], F32, kind="ExternalInput")
    w2_d = nc.dram_tensor("w2", [P, 8], F32, kind="ExternalInput")
    b2_d = nc.dram_tensor("b2", [P, 1], F32, kind="ExternalInput")
    y_out = nc.dram_tensor("y_out", [P, nt], F32, kind="ExternalOutput")

    with tile.TileContext(nc) as tc, ExitStack() as ctx:
        singles = ctx.enter_context(tc.tile_pool(name="singles", bufs=1))
        gbigT_p = ctx.enter_context(tc.tile_pool(name="gbigT", bufs=2))
        small_p = ctx.enter_context(tc.tile_pool(name="small", bufs=2))
        tp_ps = ctx.enter_context(
            tc.tile_pool(name="tp_ps", bufs=2, space=bass.MemorySpace.PSUM)
        )
        gt_ps_p = ctx.enter_context(
            tc.tile_pool(name="gt_ps", bufs=2, space=bass.MemorySpace.PSUM)
        )
        at_ps_p = ctx.enter_context(
            tc.tile_pool(name="at_ps", bufs=2, space=bass.MemorySpace.PSUM)
        )
        h_ps_p = ctx.enter_context(
            tc.tile_pool(name="h_ps", bufs=2, space=bass.MemorySpace.PSUM)
        )

        # --- constants -------------------------------------------------
        ident = singles.tile([P, P], BF16)
        masks.make_identity(nc, ident[:])
        nc.gpsimd.load_library(library_config.mlp)
        attn_w_s = singles.tile([P, 3], BF16)
        nc.sync.dma_start(out=attn_w_s[:], in_=attn_w_d.ap())
        w1_s = singles.tile([3 * EMB, 8], BF16)
        nc.sync.dma_start(out=w1_s[:], in_=w1_d.ap())
        attnb_s = singles.tile([P, 3], F32)
        nc.sync.dma_start(out=attnb_s[:], in_=attnb_d.ap())
        b1_s = singles.tile([P, 8], F32)
        nc.sync.dma_start(out=b1_s[:], in_=b1_d.ap())
        w2_s = singles.tile([P, 8], F32)
        nc.sync.dma_start(out=w2_s[:], in_=w2_d.ap())
        b2_s = singles.tile([P, 1], F32)
        nc.sync.dma_start(out=b2_s[:], in_=b2_d.ap())

        gidx_s = singles.tile([P, npos // 16], I16)
        nc.sync.dma_start(out=gidx_s[:], in_=gidx.ap())
        iidx_s = singles.tile([P, npos // 16], I16)
        nc.sync.dma_start(out=iidx_s[:], in_=iidx.ap())

        gdst = singles.tile([P, nt, P], BF16)       # group rows (+item merged)
        idst = singles.tile([P, nt, ITEM_ELEM], BF16)  # item rows
        ypre = singles.tile([P, nt], F32)

        # --- item gathers ---------------------------------------------
        split_gather(nc, idst, 0, item_tab.ap(), iidx_s, 0, npos, ITEM_ELEM)

        for k in range(N_CHUNKS):
            ksl = slice(k * tpc, (k + 1) * tpc)
            # --- group-chunk gather (256B rows) ------------------------
            lo = k * CHUNK
            hi = min((k + 1) * CHUNK, NUM_GROUPS)
            split_gather(nc, gdst, k * tpc, group_tab.ap()[lo:hi, :],
                         gidx_s, k * npc, npc, P)
            # --- merge item cols into gbig -----------------------------
            gbig = gdst[:, ksl, :]
            if ITEM_ELEM == EMB:
                nc.vector.tensor_copy(out=gbig[:, :, 3 * EMB : 4 * EMB],
                                      in_=idst[:, ksl, :])
            else:
                # padded item rows: item data in cols 96:128, zeros elsewhere
                nc.vector.tensor_tensor(out=gbig[:], in0=gbig[:],
                                        in1=idst[:, ksl, :], op=ADD)

            # --- transpose tiles: gbig -> gbigT ------------------------
            gbigT = gbigT_p.tile([P, tpc, P], BF16, tag="gbigT")
            for g0 in range(0, tpc, 4):
                gsz = min(4, tpc - g0)
                pst = tp_ps.tile([P, 4, P], BF16, tag="tp")
                for j in range(gsz):
                    nc.tensor.matmul(
                        pst[:, j, :], lhsT=gbig[:, g0 + j, :], rhs=ident[:],
                        is_transpose=True, start=True, stop=True,
                        skip_group_check=True,
                    )
                nc.vector.tensor_copy(
                    out=gbigT[:, g0 : g0 + gsz, :], in_=pst[:, 0:gsz, :]
                )

            # --- attention logits: at = gi @ attn_W + b ----------------
            at_ps = at_ps_p.tile([P, tpc, 3], F32, tag="at")
            for j in range(tpc):
                nc.tensor.matmul(
                    at_ps[:, j, :], lhsT=gbigT[:, j, :], rhs=attn_w_s[:],
                    start=True, stop=True, skip_group_check=True,
                )
            at_sb = small_p.tile([P, tpc, 3], BF16, tag="at_sb")
            nc.vector.tensor_tensor(
                out=at_sb[:], in0=at_ps[:],
                in1=attnb_s[:].unsqueeze(1).to_broadcast([P, tpc, 3]), op=ADD,
            )

            # --- g = sum_k at_k * mem_k (row-major, DVE) ---------------
            g_sb = small_p.tile([P, tpc, EMB], BF16, tag="g_sb")
            tmp0 = small_p.tile([P, tpc, EMB], BF16, tag="tmp0")
            tmp1 = small_p.tile([P, tpc, EMB], BF16, tag="tmp1")
            nc.vector.tensor_tensor(
                out=tmp0[:], in0=gbig[:, :, 0:EMB],
                in1=at_sb[:, :, 0].unsqueeze(2).to_broadcast([P, tpc, EMB]),
                op=MULT,
            )
            nc.vector.tensor_tensor(
                out=tmp1[:], in0=gbig[:, :, EMB : 2 * EMB],
                in1=at_sb[:, :, 1].unsqueeze(2).to_broadcast([P, tpc, EMB]),
                op=MULT,
            )
            nc.vector.tensor_tensor(out=tmp0[:], in0=tmp0[:], in1=tmp1[:], op=ADD)
            nc.vector.tensor_tensor(
                out=tmp1[:], in0=gbig[:, :, 2 * EMB : 3 * EMB],
                in1=at_sb[:, :, 2].unsqueeze(2).to_broadcast([P, tpc, EMB]),
                op=MULT,
            )
            nc.vector.tensor_tensor(out=g_sb[:], in0=tmp0[:], in1=tmp1[:], op=ADD)

            # --- newGT = [gitT; gT; itT] on 96 partitions --------------
            newGT = gbigT_p.tile([3 * EMB, tpc, P], BF16, tag="newGT")
            for g0 in range(0, tpc, 4):
                gsz = min(4, tpc - g0)
                sl = slice(g0, g0 + gsz)
                gt_ps = gt_ps_p.tile([EMB, 4, P], BF16, tag="gt")
                for j in range(gsz):
                    nc.tensor.matmul(
                        gt_ps[:, j, :], lhsT=g_sb[:, g0 + j, :], rhs=ident[:],
                        is_transpose=True, start=True, stop=True,
                        skip_group_check=True,
                    )
                # gT -> partitions 32:64
                nc.scalar.copy(out=newGT[EMB : 2 * EMB, sl, :],
                               in_=gt_ps[:, 0:gsz, :])
                # gitT = gT * itT -> partitions 0:32
                nc.vector.tensor_tensor(
                    out=newGT[0:EMB, sl, :], in0=gt_ps[:, 0:gsz, :],
                    in1=gbigT[3 * EMB : 4 * EMB, sl, :], op=MULT,
                )
                # itT -> partitions 64:96
                nc.scalar.copy(
                    out=newGT[2 * EMB : 3 * EMB, sl, :],
                    in_=gbigT[3 * EMB : 4 * EMB, sl, :],
                )

            # --- h = relu(new @ W1 + b1) -------------------------------
            h_ps = h_ps_p.tile([P, tpc, 8], F32, tag="h")
            for j in range(tpc):
                nc.tensor.matmul(
                    h_ps[:, j, :], lhsT=newGT[:, j, :], rhs=w1_s[:],
                    start=True, stop=True, skip_group_check=True,
                )
            h_sb = small_p.tile([P, tpc, 8], F32, tag="h_sb")
            nc.vector.tensor_tensor(
                out=h_sb[:], in0=h_ps[:],
                in1=b1_s[:].unsqueeze(1).to_broadcast([P, tpc, 8]), op=ADD,
            )
            nc.vector.tensor_scalar_max(h_sb[:], h_sb[:], 0.0)

            # --- y = h @ W2 --------------------------------------------
            hw = small_p.tile([P, tpc, 8], F32, tag="hw")
            nc.vector.tensor_tensor(
                out=hw[:], in0=h_sb[:],
                in1=w2_s[:].unsqueeze(1).to_broadcast([P, tpc, 8]), op=MULT,
            )
            nc.vector.tensor_reduce(
                out=ypre[:, ksl], in_=hw[:], axis=AXX, op=ADD
            )

        # --- sigmoid(y + b2) and store --------------------------------
        ysig = singles.tile([P, nt], F32)
        nc.scalar.activation(
            out=ysig[:], in_=ypre[:],
            func=mybir.ActivationFunctionType.Sigmoid,
            bias=b2_s[:, 0:1], scale=1.0,
        )
        nc.sync.dma_start(out=y_out.ap(), in_=ysig[:])

    nc.compile()
    return nc


def wrap_idx(vals, npos):
    """[n] -> [128, npos//16] int16: position j -> partition j%16 (replicated
    across the 8 16-partition groups), column j//16."""
    full = np.zeros(npos, np.int16)
    full[: len(vals)] = vals
    block = full.reshape(npos // 16, 16).T
    return np.ascontiguousarray(np.tile(block, (8, 1)))


def prep_host_inputs(inputs, n_cores=N_CORES):
    """Tables (bf16), per-core chunk-sorted index layouts, output scatter map."""
    grp = np.asarray(inputs["group_inputs"]).astype(np.int64).reshape(-1)
    itm = np.asarray(inputs["item_inputs"]).astype(np.int64).reshape(-1)
    nq = grp.shape[0]

    user_emb = np.asarray(inputs["user_emb"], np.float32)
    members = np.asarray(inputs["members"]).astype(np.int64)
    group_tab = np.zeros((members.shape[0], P), ml_dtypes.bfloat16)
    group_tab[:, : 3 * EMB] = (
        user_emb[members.reshape(-1)]
        .reshape(members.shape[0], 3 * EMB)
        .astype(ml_dtypes.bfloat16)
    )
    item_rows = np.concatenate(
        [
            np.asarray(inputs["item_emb"], np.float32),
            np.asarray(inputs["genres"], np.float32),
        ],
        axis=1,
    ).astype(ml_dtypes.bfloat16)
    if ITEM_ELEM == EMB:
        item_tab = np.ascontiguousarray(item_rows)
    else:
        item_tab = np.zeros((NUM_ITEMS, ITEM_ELEM), ml_dtypes.bfloat16)
        item_tab[:, 3 * EMB : 4 * EMB] = item_rows

    # --- assign queries to cores by item range, sort by group chunk ----
    core_of = itm // ITEMS_PER_CORE
    per_core = []  # (perm_chunks: list of global query idx arrays per chunk)
    max_nk = 1
    for c in range(n_cores):
        qc = np.nonzero(core_of == c)[0]
        chunk = grp[qc] // CHUNK
        chunks = [qc[chunk == k] for k in range(N_CHUNKS)]
        per_core.append(chunks)
        if len(qc):
            max_nk = max(max_nk, max(len(x) for x in chunks))
    npc = -(-max_nk // P) * P  # round up to 128
    npos = N_CHUNKS * npc

    in_extra = []
    perms = []
    for c in range(n_cores):
        gl = np.zeros(npos, np.int16)
        il = np.zeros(npos, np.int16)
        pm = np.full(npos, -1, np.int64)
        for k, qk in enumerate(per_core[c]):
            o = k * npc
            n = len(qk)
            gl[o : o + n] = (grp[qk] - k * CHUNK).astype(np.int16)
            il[o : o + n] = (itm[qk] - c * ITEMS_PER_CORE).astype(np.int16)
            pm[o : o + n] = qk
        in_extra.append(
            {
                "gidx": wrap_idx(gl, npos),
                "iidx": wrap_idx(il, npos),
                "item_tab": np.ascontiguousarray(
                    item_tab[c * ITEMS_PER_CORE : (c + 1) * ITEMS_PER_CORE]
                ),
            }
        )
        perms.append(pm)

    attn_W = np.asarray(inputs["attn_W"], np.float32)
    attn_b = np.asarray(inputs["attn_b"], np.float32)
    w1 = np.asarray(inputs["pred_W1"], np.float32)
    b1 = np.asarray(inputs["pred_b1"], np.float32)
    w2 = np.asarray(inputs["pred_W2"], np.float32)
    b2 = np.asarray(inputs["pred_b2"], np.float32)
    weights = {
        "attn_w": np.ascontiguousarray(attn_W.astype(ml_dtypes.bfloat16)),
        "w1": np.ascontiguousarray(w1.astype(ml_dtypes.bfloat16)),
        "attnb": np.ascontiguousarray(np.tile(attn_b[None, :], (P, 1))),
        "b1": np.ascontiguousarray(np.tile(b1[None, :], (P, 1))),
        "w2": np.ascontiguousarray(np.tile(w2[:, 0][None, :], (P, 1))),
        "b2": np.ascontiguousarray(np.tile(b2.reshape(1, 1), (P, 1))),
    }
    return group_tab, weights, in_extra, perms, npc, nq


def make_in_maps(group_tab, weights, in_extra):
    return [{"group_tab": group_tab, **weights, **ex} for ex in in_extra]


_NC_CACHE = {}


def kernel(**inputs) -> np.ndarray:
    group_tab, weights, in_extra, perms, npc, nq = prep_host_inputs(inputs)
    if npc not in _NC_CACHE:
        _NC_CACHE[npc] = build(npc)
    nc = _NC_CACHE[npc]
    in_maps = make_in_maps(group_tab, weights, in_extra)
    res = run_bass_kernel_spmd(nc, in_maps, core_ids=list(range(N_CORES)))
    y = np.zeros(nq, np.float32)
    for c in range(N_CORES):
        yc = res.results[c]["y_out"]          # [128, nt]; position j -> [j%128, j//128]
        flat = np.ascontiguousarray(yc.T).reshape(-1)
        pm = perms[c]
        valid = pm >= 0
        y[pm[valid]] = flat[valid]
    return y.reshape(-1, 1).astype(np.float32)



# revision 7
# speedup vs baseline: 3.2915x; 3.2915x over previous
"""Trainium2 Bass kernel for the BAHDANAU+ group-recommendation model (v4).

kernel(**inputs) takes the complete (unsharded) numpy inputs, distributes the
131072-query batch over 8 NeuronCores, runs the Bass kernel SPMD, and returns
the full [B, 1] float32 output.

Architecture (v4):
  Host-side TABLE transforms (all query-independent):
    group_tab[g] (256B rows, bf16): cols 0:96 = user_emb[members[g]] flat,
    96:99 = A_g = mem_flat @ attn_W[0:96] + attn_b, 99:123 = R_g =
    per-member mem_k @ pred_W1[32:64] ([3,8] k-major).  item_tab[i] (256B
    stride, 128B payload): cols 0:32 = item_emb||genres, 32:35 = B_i =
    it @ attn_W[96:128], 35:43 = Q_i = it @ pred_W1[64:96] + pred_b1.
    With these, at = A_g + B_i (attention logits as precomputed linear
    partials) and two of the three pred_W1 terms become DVE adds.

  Sharding: queries -> cores by GROUP range (62500 groups/core).  One
  dma_gather window with an idx-32768 base-slide covers the whole per-core
  group slice (signed-idx addressing in the SWDGE ucode), so positions need
  no group ordering.  Positions are sorted by ITEM id and segmented into 4
  contiguous item-quarter windows of 25000 rows (positive int16 indices).
  The ucode drops TRAILING negative indices of each gather, so host prep
  guarantees the last real position of every group-gather piece has a
  non-negative (rebased) index (swap or sentinel).

  Gathers: SWDGE dma_gather on FOUR queues (concurrent Q7 core pairs;
  measured ~3.8 ns/idx at 4 queues vs 11.5 serial).  512-idx pieces; group
  rows 256B, item rows 128B payload at 256B stride (direct InstDMAGatherAnt
  emission to relax the elem%256B assert).

  Compute (row-major): at = A_g+B_i; g = sum_k at_k mem_k; z = g*it;
  gw1b = at . R_g + Q_i (DVE); per tile on PE: transpose z, h8 = zT^T@W1a;
  h = relu(h8 + gw1b); y = sigmoid(sum h*W2 + b2).

  Output y is in permuted position order; the host scatters it back.
"""

import sys

sys.path.insert(0, "/opt/trn_rl_repo")

from contextlib import ExitStack

import numpy as np
import ml_dtypes

import concourse.bacc as bacc
import concourse.bass as bass
import concourse.tile as tile
from concourse import library_config, masks, mybir
from concourse.ap_utils import ap_is_contiguous
from concourse.bass_utils import run_bass_kernel_spmd

N_CORES = 8
P = 128
EMB = 32
B = 131_072
NUM_USERS = 1_000_000
NUM_ITEMS = 100_000
NUM_GROUPS = 500_000
GPC = NUM_GROUPS // N_CORES        # groups per core (62500 < 65536)
IQ = NUM_ITEMS // 4                # item quarter-window (25000 < 32768)
SLIDE = 32_768                     # group idx base-slide
PIECE_T = 4                        # gather piece size in tiles (512 idx)
NQ = 4                             # SWDGE queues
CBLK = 8                           # compute-block tiles

GCOLS = 128                        # group row cols (bf16) = 256B
ICOLS = 64                         # item row payload cols = 128B
ISTEP = 128                        # item row stride cols = 256B

F32 = mybir.dt.float32
BF16 = mybir.dt.bfloat16
I16 = mybir.dt.int16
MULT = mybir.AluOpType.mult
ADD = mybir.AluOpType.add
AXX = mybir.AxisListType.X


def emit_gather(gp, out_ap, in_ap, idxs_ap, num_idxs, elem_size, queue_num,
                elem_step=None):
    """BassGpSimd.dma_gather (non-transpose, DRAM src) without the elem%256B
    assert; elem_step = row stride in elements (stride bytes must be %256)."""
    assert idxs_ap.dtype == mybir.dt.int16
    assert in_ap.dtype == out_ap.dtype
    assert in_ap.space == bass.MemorySpace.DRAM
    assert idxs_ap.space == bass.MemorySpace.SBUF
    assert out_ap.space == bass.MemorySpace.SBUF
    assert ap_is_contiguous(in_ap.ap[1:])
    assert ap_is_contiguous(out_ap.ap[1:])
    assert ap_is_contiguous(idxs_ap.ap[1:])
    if elem_step is None:
        elem_step = elem_size
    assert out_ap.ap[-1][1] == elem_size
    assert in_ap.ap[0][0] == elem_step
    assert out_ap.ap[0][1] * out_ap.ap[1][1] == ((num_idxs + P - 1) // P) * P
    stride_bytes = elem_step * mybir.dt.size(in_ap.dtype)
    assert stride_bytes % 256 == 0 and stride_bytes // 256 < 256
    _in_ap = gp.lower_ap_dma(in_ap, for_custom_bir_dma=True)
    inst = gp.add_instruction(
        mybir.InstDMAGatherAnt(
            name=gp.bass.get_next_instruction_name(),
            ins=[*_in_ap, gp.lower_ap(idxs_ap),
                 gp.lower_val_access(gp.to_reg(num_idxs))],
            outs=[gp.lower_ap(out_ap)],
            transpose=False,
            num_idxs=num_idxs,
            elem_size=elem_size,
            stride_bytes_256=stride_bytes // 256,
            gen_mode=0,
            single_packet=True,
            queue_num=queue_num,
            sbuf_tokens_per_rank=0,
            sbuf_free_dim_per_rank=0,
            sbuf_free_dim_pad_per_rank=0,
            sbuf_byte_offset=0,
        )
    )
    return inst.annotate(f"swdge_q={queue_num}")


def seg_pieces(ts):
    """Tile-ranges of gather pieces within one ts-tile segment."""
    out = []
    t = 0
    while t < ts:
        n = min(PIECE_T, ts - t)
        out.append((t, t + n))
        t += n
    return out


def build(ts, gathers_only=False):
    """Per-core program; ts = tiles per item-quarter segment (even)."""
    nt = 4 * ts
    assert nt % CBLK == 0
    npos = nt * P
    nblk = nt // CBLK

    nc = bacc.Bacc(
        "TRN2",
        target_bir_lowering=False,
        debug=False,
        enable_asserts=False,
        num_swdge_queues=NQ,
    )

    gidx = nc.dram_tensor("gidx", [P, npos // 16], I16, kind="ExternalInput")
    iidx = nc.dram_tensor("iidx", [P, npos // 16], I16, kind="ExternalInput")
    gslice = nc.dram_tensor("gslice", [GPC, GCOLS], BF16, kind="ExternalInput")
    item_tab = nc.dram_tensor("item_tab", [NUM_ITEMS, ISTEP], BF16,
                              kind="ExternalInput")
    w1a_d = nc.dram_tensor("w1a", [EMB, 8], BF16, kind="ExternalInput")
    w2_d = nc.dram_tensor("w2", [P, 8], F32, kind="ExternalInput")
    b2_d = nc.dram_tensor("b2", [P, 1], F32, kind="ExternalInput")
    y_out = nc.dram_tensor("y_out", [P, nt], F32, kind="ExternalOutput")

    with tile.TileContext(nc) as tc, ExitStack() as ctx:
        singles = ctx.enter_context(tc.tile_pool(name="singles", bufs=1))
        dve_p = ctx.enter_context(tc.tile_pool(name="dve", bufs=2))
        zt_p = ctx.enter_context(tc.tile_pool(name="zt", bufs=2))
        tp_ps = ctx.enter_context(
            tc.tile_pool(name="tp_ps", bufs=2, space=bass.MemorySpace.PSUM)
        )
        h_ps_p = ctx.enter_context(
            tc.tile_pool(name="h_ps", bufs=2, space=bass.MemorySpace.PSUM)
        )

        # --- constants -------------------------------------------------
        ident = singles.tile([P, P], BF16)
        masks.make_identity(nc, ident[:])
        nc.gpsimd.load_library(library_config.mlp)
        w1a_s = singles.tile([EMB, 8], BF16)
        nc.sync.dma_start(out=w1a_s[:], in_=w1a_d.ap())
        w2_s = singles.tile([P, 8], F32)
        nc.sync.dma_start(out=w2_s[:], in_=w2_d.ap())
        b2_s = singles.tile([P, 1], F32)
        nc.sync.dma_start(out=b2_s[:], in_=b2_d.ap())
        gidx_s = singles.tile([P, npos // 16], I16)
        nc.sync.dma_start(out=gidx_s[:], in_=gidx.ap())
        iidx_s = singles.tile([P, npos // 16], I16)
        nc.sync.dma_start(out=iidx_s[:], in_=iidx.ap())

        gdst = singles.tile([P, nt, GCOLS], BF16)
        idst = singles.tile([P, nt, ICOLS], BF16)
        ypre = singles.tile([P, nt], F32)

        # --- gathers: pieces over 4 queues -----------------------------
        g_base = gslice.ap()[SLIDE:, :]
        qn = 0
        for k in range(4):
            i_base = item_tab.ap()[k * IQ:, :]
            for (a, b) in seg_pieces(ts):
                t0, t1 = k * ts + a, k * ts + b
                n_idx = (t1 - t0) * P
                emit_gather(nc.gpsimd, gdst[:, t0:t1, :], g_base,
                            gidx_s[:, t0 * 8:t1 * 8], n_idx, GCOLS, qn % NQ)
                qn += 1
                emit_gather(nc.gpsimd, idst[:, t0:t1, :], i_base,
                            iidx_s[:, t0 * 8:t1 * 8], n_idx, ICOLS, qn % NQ,
                            elem_step=ISTEP)
                qn += 1

        # --- compute per block of CBLK tiles ---------------------------
        for blk in range(nblk):
            sl = slice(blk * CBLK, (blk + 1) * CBLK)
            gb = gdst[:, sl, :]
            ib = idst[:, sl, :]

            if gathers_only:
                nc.vector.tensor_reduce(out=ypre[:, sl], in_=gb[:, :, 0:8],
                                        axis=AXX, op=ADD)
                continue

            # at = A_g + B_i  [P, CBLK, 3]
            at = dve_p.tile([P, CBLK, 3], BF16, tag="at")
            nc.vector.tensor_tensor(out=at[:], in0=gb[:, :, 96:99],
                                    in1=ib[:, :, 32:35], op=ADD)

            # g = sum_k at_k * mem_k  [P, CBLK, 32]
            g_t = dve_p.tile([P, CBLK, EMB], BF16, tag="g")
            tmp = dve_p.tile([P, CBLK, EMB], BF16, tag="tmp")
            nc.vector.tensor_tensor(
                out=g_t[:], in0=gb[:, :, 0:EMB],
                in1=at[:, :, 0].unsqueeze(2).to_broadcast([P, CBLK, EMB]),
                op=MULT)
            nc.vector.tensor_tensor(
                out=tmp[:], in0=gb[:, :, EMB:2 * EMB],
                in1=at[:, :, 1].unsqueeze(2).to_broadcast([P, CBLK, EMB]),
                op=MULT)
            nc.vector.tensor_tensor(out=g_t[:], in0=g_t[:], in1=tmp[:], op=ADD)
            nc.vector.tensor_tensor(
                out=tmp[:], in0=gb[:, :, 2 * EMB:3 * EMB],
                in1=at[:, :, 2].unsqueeze(2).to_broadcast([P, CBLK, EMB]),
                op=MULT)
            nc.vector.tensor_tensor(out=g_t[:], in0=g_t[:], in1=tmp[:], op=ADD)

            # z = g * it  [P, CBLK, 32]
            z_t = dve_p.tile([P, CBLK, EMB], BF16, tag="z")
            nc.vector.tensor_tensor(out=z_t[:], in0=g_t[:],
                                    in1=ib[:, :, 0:EMB], op=MULT)

            # gw1b = at . R_g + Q_i  [P, CBLK, 8]
            gw = dve_p.tile([P, CBLK, 8], F32, tag="gw")
            tm8 = dve_p.tile([P, CBLK, 8], F32, tag="tm8")
            nc.vector.tensor_tensor(
                out=gw[:], in0=gb[:, :, 99:107],
                in1=at[:, :, 0].unsqueeze(2).to_broadcast([P, CBLK, 8]),
                op=MULT)
            nc.vector.tensor_tensor(
                out=tm8[:], in0=gb[:, :, 107:115],
                in1=at[:, :, 1].unsqueeze(2).to_broadcast([P, CBLK, 8]),
                op=MULT)
            nc.vector.tensor_tensor(out=gw[:], in0=gw[:], in1=tm8[:], op=ADD)
            nc.vector.tensor_tensor(
                out=tm8[:], in0=gb[:, :, 115:123],
                in1=at[:, :, 2].unsqueeze(2).to_broadcast([P, CBLK, 8]),
                op=MULT)
            nc.vector.tensor_tensor(out=gw[:], in0=gw[:], in1=tm8[:], op=ADD)
            nc.vector.tensor_tensor(out=gw[:], in0=gw[:], in1=ib[:, :, 35:43],
                                    op=ADD)

            # PE: zT per tile, then h8 = zT^T @ W1a
            h_ps = h_ps_p.tile([P, CBLK, 8], F32, tag="h")
            for g0 in range(0, CBLK, 4):
                pst = tp_ps.tile([EMB, 4, P], BF16, tag="tp")
                for j in range(4):
                    nc.tensor.matmul(
                        pst[:, j, :], lhsT=z_t[:, g0 + j, :], rhs=ident[:],
                        is_transpose=True, start=True, stop=True,
                        skip_group_check=True,
                    )
                zt_sb = zt_p.tile([EMB, 4, P], BF16, tag="zt")
                nc.scalar.copy(out=zt_sb[:], in_=pst[:])
                for j in range(4):
                    nc.tensor.matmul(
                        h_ps[:, g0 + j, :], lhsT=zt_sb[:, j, :], rhs=w1a_s[:],
                        start=True, stop=True, skip_group_check=True,
                    )

            # h = relu(h8 + gw)  [P, CBLK, 8]
            h_sb = dve_p.tile([P, CBLK, 8], F32, tag="h_sb")
            nc.vector.tensor_tensor(out=h_sb[:], in0=h_ps[:], in1=gw[:], op=ADD)
            nc.vector.tensor_scalar_max(h_sb[:], h_sb[:], 0.0)

            # y = sum h * w2
            hw = dve_p.tile([P, CBLK, 8], F32, tag="hw")
            nc.vector.tensor_tensor(
                out=hw[:], in0=h_sb[:],
                in1=w2_s[:].unsqueeze(1).to_broadcast([P, CBLK, 8]), op=MULT)
            nc.vector.tensor_reduce(out=ypre[:, sl], in_=hw[:], axis=AXX,
                                    op=ADD)

        # --- sigmoid(y + b2) and store --------------------------------
        ysig = singles.tile([P, nt], F32)
        nc.scalar.activation(
            out=ysig[:], in_=ypre[:],
            func=mybir.ActivationFunctionType.Sigmoid,
            bias=b2_s[:, 0:1], scale=1.0,
        )
        nc.sync.dma_start(out=y_out.ap(), in_=ysig[:])

    nc.compile()
    return nc


def wrap_idx(vals):
    """[npos] -> [128, npos//16] int16: position j -> partition j%16
    (replicated across the 8 16-partition groups), column j//16."""
    npos = len(vals)
    block = vals.reshape(npos // 16, 16).T
    return np.ascontiguousarray(np.tile(block, (8, 1)))


def prep_host_inputs(inputs, n_cores=N_CORES):
    grp = np.asarray(inputs["group_inputs"]).astype(np.int64).reshape(-1)
    itm = np.asarray(inputs["item_inputs"]).astype(np.int64).reshape(-1)
    nq = grp.shape[0]

    user_emb = np.asarray(inputs["user_emb"], np.float32)
    members = np.asarray(inputs["members"]).astype(np.int64)
    attn_W = np.asarray(inputs["attn_W"], np.float32)
    attn_b = np.asarray(inputs["attn_b"], np.float32)
    w1 = np.asarray(inputs["pred_W1"], np.float32)
    b1 = np.asarray(inputs["pred_b1"], np.float32)
    w2 = np.asarray(inputs["pred_W2"], np.float32)
    b2 = np.asarray(inputs["pred_b2"], np.float32)

    # --- group table: mem | A_g | R_g ---------------------------------
    mem_flat = user_emb[members.reshape(-1)].reshape(NUM_GROUPS, 3 * EMB)
    group_tab = np.zeros((NUM_GROUPS, GCOLS), ml_dtypes.bfloat16)
    group_tab[:, :3 * EMB] = mem_flat.astype(ml_dtypes.bfloat16)
    a_g = mem_flat @ attn_W[:3 * EMB] + attn_b[None, :]        # [G, 3]
    group_tab[:, 96:99] = a_g.astype(ml_dtypes.bfloat16)
    w1b = w1[EMB:2 * EMB]                                      # [32, 8]
    r_g = np.einsum("gkc,cj->gkj",
                    mem_flat.reshape(NUM_GROUPS, 3, EMB), w1b)  # [G, 3, 8]
    group_tab[:, 99:123] = r_g.reshape(NUM_GROUPS, 24).astype(ml_dtypes.bfloat16)

    # --- item table: it | B_i | Q_i -----------------------------------
    it_rows = np.concatenate(
        [np.asarray(inputs["item_emb"], np.float32),
         np.asarray(inputs["genres"], np.float32)], axis=1)     # [I, 32]
    item_tab = np.zeros((NUM_ITEMS, ISTEP), ml_dtypes.bfloat16)
    item_tab[:, :EMB] = it_rows.astype(ml_dtypes.bfloat16)
    b_i = it_rows @ attn_W[3 * EMB:]                            # [I, 3]
    item_tab[:, 32:35] = b_i.astype(ml_dtypes.bfloat16)
    q_i = it_rows @ w1[2 * EMB:] + b1[None, :]                  # [I, 8]
    item_tab[:, 35:43] = q_i.astype(ml_dtypes.bfloat16)

    # --- assign queries to cores by group range; item-sorted segments --
    core_of = grp // GPC
    per_core = []
    max_seg = 1
    for c in range(n_cores):
        qc = np.nonzero(core_of == c)[0]
        qc = qc[np.argsort(itm[qc], kind="stable")]
        bounds = np.searchsorted(itm[qc], [0, IQ, 2 * IQ, 3 * IQ, NUM_ITEMS])
        segs = [qc[bounds[k]:bounds[k + 1]] for k in range(4)]
        per_core.append(segs)
        max_seg = max(max_seg, max(len(s) for s in segs))
    ts = -(-max_seg // P)
    ts += ts % 2                       # even -> nt % CBLK == 0
    npos = 4 * ts * P

    in_extra = []
    perms = []
    for c in range(n_cores):
        # Pads use index 0 (a valid row), NEVER negative: the ucode
        # self-trims trailing negative indices, which desyncs its
        # descriptor count from the decode-side ring reservation and
        # corrupts the SWDGE ring once it wraps (device fault).
        gl = np.zeros(npos, np.int16)
        il = np.zeros(npos, np.int16)
        pm = np.full(npos, -1, np.int64)
        for k, qs in enumerate(per_core[c]):
            o = k * ts * P
            n = len(qs)
            gl[o:o + n] = (grp[qs] - c * GPC - SLIDE).astype(np.int16)
            il[o:o + n] = (itm[qs] - k * IQ).astype(np.int16)
            pm[o:o + n] = qs
            # last position of each piece must have gidx >= 0 (no trim)
            for (a, b) in seg_pieces(ts):
                last = o + b * P - 1
                if gl[last] >= 0:
                    continue
                p0 = o + a * P
                cand = np.nonzero(gl[p0:last] >= 0)[0]
                assert len(cand), "all-negative gather piece"
                j = p0 + cand[-1]
                for arr in (gl, il, pm):
                    arr[j], arr[last] = arr[last], arr[j]
        in_extra.append({"gidx": wrap_idx(gl), "iidx": wrap_idx(il)})
        perms.append(pm)

    gslices = [np.ascontiguousarray(group_tab[c * GPC:(c + 1) * GPC])
               for c in range(n_cores)]
    weights = {
        "item_tab": item_tab,
        "w1a": np.ascontiguousarray(w1[:EMB].astype(ml_dtypes.bfloat16)),
        "w2": np.ascontiguousarray(np.tile(w2[:, 0][None, :], (P, 1))),
        "b2": np.ascontiguousarray(np.tile(b2.reshape(1, 1), (P, 1))),
    }
    return gslices, weights, in_extra, perms, ts, nq


def make_in_maps(gslices, weights, in_extra):
    return [{"gslice": gslices[c], **weights, **ex}
            for c, ex in enumerate(in_extra)]


_NC_CACHE = {}


def kernel(**inputs) -> np.ndarray:
    gslices, weights, in_extra, perms, ts, nq = prep_host_inputs(inputs)
    if ts not in _NC_CACHE:
        _NC_CACHE[ts] = build(ts)
    nc = _NC_CACHE[ts]
    in_maps = make_in_maps(gslices, weights, in_extra)
    res = run_bass_kernel_spmd(nc, in_maps, core_ids=list(range(N_CORES)))
    y = np.zeros(nq, np.float32)
    for c in range(N_CORES):
        yc = res.results[c]["y_out"]   # [128, nt]; position j -> [j%128, j//128]
        flat = np.ascontiguousarray(yc.T).reshape(-1)
        pm = perms[c]
        valid = pm >= 0
        y[pm[valid]] = flat[valid]
    return y.reshape(-1, 1).astype(np.float32)
